# revision 11
# baseline (speedup 1.0000x reference)
"""Trainium2 Bass kernel for nn_EqvRESFeedForward (gnn_message_passing).

Strategy (v2.1)
---------------
The reference computes, twice, an e3nn-style radial convolution
    out[b,n,i] = (1/sqrt(N)) * sum_m R(r_bnm)[i,:] @ x[b,m,:]
with R(r) = reshape(swish(rbf(r) @ W1) @ W2, [C,C]).  The composite map
r -> R(r) is a family of C*C smooth scalar functions of one variable.
At call time (host, numpy) we refit that family onto D Gaussians IN
s = r^2 SPACE,
    phi_d(s) = exp(-((s - sc_d)/sw_d)^2),
weighted by the *empirical* pair-distance distribution (the diagonal
r=0 self-pairs get a coherent-error spike weight).  The device then
evaluates each basis function with a SINGLE ScalarE instruction:
AF.Derivative_Erf(x) = (2/sqrt(pi)) exp(-x^2) applied to s with
per-partition scale/bias (2/sqrt(pi) and 1/sqrt(N) fold into the
fitted coefficients).  s comes straight from one K=18 bf16 matmul
(3-level hi/lo split, near-f32 accurate), so there is no Ln/Exp/sqrt
chain and no VectorE arithmetic in the basis at all.  The pairwise
datapath (basis, z, conv matmuls) runs in fp16, which matmuls at the
same 1 cycle/row as bf16 but with 8x finer mantissa.

Sharding: the m (source-node) axis is split across the 8 cores (48
each).  Each core computes partial conv1 outputs for all (b, n); a
per-b ReduceScatter hands each core its m-slice of the full conv1
output, and a tiny AllReduce at the end sums the masked node
reduction.  The normalize/fc2/softmax tail runs redundantly on every
core in a [C, B] layout using GpSimd partition-reductions, PE
partition-broadcasts, a bit-trick+Newton rsqrt and a repeated-squaring
exp, so the ScalarE activation table never swaps away from
Derivative_Erf and the Vector engine stays nearly free for the conv
copies.

Device layout: partitions p = (dlo, m') with dlo = d%2, m' = m-slice
index padded 48->64.  The two batches' s matrices live side by side in
one PSUM tile so each of the DH = D/2 Derivative_Erf instructions
covers both batches (halving ScalarE's fixed per-instruction cost).
"""
import os
import sys
import time

import numpy as np

for _p in ("/opt/trn_rl_repo", "/root/.axon_site/_ro/trn_rl_repo"):
    if os.path.isdir(_p) and _p not in sys.path:
        sys.path.insert(0, _p)

import concourse.bacc as bacc
import concourse.bass as bass
import concourse.mybir as mybir
import concourse.tile as tile
from concourse.bass_utils import run_bass_kernel_spmd

# ---- problem constants (hardcoded per contract) ----
B, N, C = 2, 384, 16
NB, H = 10, 64
MAX_RADIUS = 10.0
N_CORES = 8
MS = N // N_CORES          # m-slice per core = 48
MP = 64                    # m padded to 64 (two d-parities -> 128 partitions)

# ---- s-space basis (centers/widths tuned offline; Q refit at call time) ----
D = 14
DH = D // 2
_RC = np.linspace(0.0, 10.4, D)
SC = (_RC * _RC).astype(np.float64)                       # centers in s
SW = (2.0 * 1.0 * np.maximum(_RC, 0.8)).astype(np.float64)  # widths in s
FIT_GRID = 4096
FIT_W0 = 40.0              # weight spike at r=0 (diagonal self-pairs)
FIT_LAM = 1e-9

KG = 18                    # K rows of the split-bf16 r^2 matmul

AF = mybir.ActivationFunctionType
ALU = mybir.AluOpType
AX = mybir.AxisListType
F32 = mybir.dt.float32
BF16 = mybir.dt.bfloat16
FP16 = mybir.dt.float16
I32 = mybir.dt.int32

_CACHE = {}

RSQRT_MAGIC = 0x5F3759DF
SQ15 = float(np.sqrt(15.0))    # sqrt(C - 1)


# ----------------------------------------------------------------------
# host-side prep (numpy; only O(N)/O(weights) work — no pairwise compute)
# ----------------------------------------------------------------------

def _bf16(a):
    import ml_dtypes
    return np.asarray(a, np.float32).astype(ml_dtypes.bfloat16)


def _bf16f(a):
    return _bf16(a).astype(np.float32)


def _fp16(a):
    return np.asarray(a, np.float32).astype(np.float16)


def _target(rv, w1, w2):
    rbf = np.exp(-((rv[:, None] - np.linspace(0.0, MAX_RADIUS, NB)) ** 2))
    pre = rbf @ w1.astype(np.float64)
    hid = pre / (1.0 + np.exp(-pre))
    return hid @ w2.astype(np.float64)


def _fit_q(w1, w2):
    """Weighted-grid LSQ refit of r -> swish(rbf(r)@w1)@w2 onto the
    s-Gaussians (w0 spike at r=0: the diagonal self-pairs contribute
    coherently across nodes)."""
    rmax = MAX_RADIUS * np.sqrt(3.0) + 0.2
    rv = np.linspace(0.0, rmax, FIT_GRID)
    sv = rv * rv
    phi = np.exp(-(((sv[:, None] - SC) / SW) ** 2))
    t = _target(rv, w1, w2)
    w = np.ones(FIT_GRID)
    w[0] = FIT_W0
    pw = np.sqrt(w)[:, None] * phi
    a = pw.T @ pw + FIT_LAM * np.eye(D)
    return np.linalg.solve(a, pw.T @ (np.sqrt(w)[:, None] * t))  # [D, C*C]


def _host_prep(x, xyz, mask, conv1_w1, conv1_w2, conv2_w1, conv2_w2, fc2_w):
    x = np.asarray(x, np.float32)
    xyz64 = np.asarray(xyz, np.float64)
    mask = np.asarray(mask)
    diag = np.einsum('bnn->bn', mask)
    keep = (diag != 0).astype(np.float32)                       # [B, N]

    # fitted coefficients; fold in sqrt(pi)/2 (DErf) and 1/sqrt(N)
    fold = np.sqrt(np.pi) / 2.0 / np.sqrt(np.float64(N))
    qs = [(_fit_q(np.asarray(conv1_w1), np.asarray(conv1_w2)) * fold),
          (_fit_q(np.asarray(conv2_w1), np.asarray(conv2_w2)) * fold)]

    # qeo[l, par][j, t*C+i] = Q_l[2t+par].reshape(C,C)[i, j]
    qeo = np.zeros((2, 2, C, DH * C), np.float32)
    for l, q in enumerate(qs):
        qr = q.reshape(D, C, C)                                 # [d, i, j]
        for par in range(2):
            qeo[l, par] = np.transpose(qr[par::2], (2, 0, 1)).reshape(C, DH * C)

    # per-partition DErf scale/bias: d = 2*t + (p // MP)
    cbP = np.zeros((128, 2 * DH), np.float32)
    for p in range(128):
        dsel = 2 * np.arange(DH) + p // MP
        cbP[p, 0:DH] = 1.0 / SW[dsel]
        cbP[p, DH:2 * DH] = -SC[dsel] / SW[dsel]

    # hi/lo splits for the r^2 matmul
    xh = _bf16f(xyz64)
    xl = _bf16f(xyz64 - xh)
    n2 = np.sum(xyz64 * xyz64, axis=-1)
    n2h = _bf16f(n2)
    n2m = _bf16f(n2 - n2h)
    n2l = _bf16f(n2 - n2h - n2m)

    # grh [B, KG, N] (shared): rhs rows
    grh = np.zeros((B, KG, N), np.float32)
    for b in range(B):
        grh[b, 0:3] = xh[b].T
        grh[b, 3:6] = xh[b].T
        grh[b, 6:9] = xl[b].T
        grh[b, 9:12] = xl[b].T
        grh[b, 12:15] = 1.0
        grh[b, 15] = n2h[b]
        grh[b, 16] = n2m[b]
        grh[b, 17] = n2l[b]

    xk = x * keep[:, :, None]                                   # masked conv1 input

    fc2t = np.ascontiguousarray(np.asarray(fc2_w, np.float32).T)

    in_maps = []
    for c in range(N_CORES):
        sl = slice(c * MS, (c + 1) * MS)
        # glh [B, KG, 128]: lhsT columns p=(par, m')
        glh = np.zeros((B, KG, 128), np.float32)
        for b in range(B):
            col = np.zeros((KG, MP), np.float32)
            col[0:3, :MS] = -2.0 * xh[b, sl].T
            col[3:6, :MS] = -2.0 * xl[b, sl].T
            col[6:9, :MS] = -2.0 * xh[b, sl].T
            col[9:12, :MS] = -2.0 * xl[b, sl].T
            col[12, :MS] = n2h[b, sl]
            col[13, :MS] = n2m[b, sl]
            col[14, :MS] = n2l[b, sl]
            col[12, MS:] = 1.0          # padded cols: s = 1
            col[15:18, :MS] = 1.0
            glh[b] = np.concatenate([col, col], axis=1)

        # bf16 blob [18, FB]: (glh|grh per b)
        FB = 2 * 512
        blobB = np.zeros((18, FB), np.float32)
        for b in range(B):
            blobB[:, b * 512: b * 512 + 128] = glh[b]
            blobB[:, b * 512 + 128: (b + 1) * 512] = grh[b]

        # fp16 blob [18, FH]: x0T (per b) | qeo (rows 0:16)
        FH = 2 * MP + 4 * DH * C
        blobH = np.zeros((18, FH), np.float32)
        x0t = np.zeros((B, C, MP), np.float32)
        x0t[:, :, :MS] = np.transpose(xk[:, sl, :], (0, 2, 1))
        for b in range(B):
            blobH[0:C, b * MP: (b + 1) * MP] = x0t[b]
        for l in range(2):
            for par in range(2):
                o = 2 * MP + (2 * l + par) * DH * C
                blobH[0:C, o: o + DH * C] = qeo[l, par]

        # f32 blob [16, FF]: keep16 (b0|b1) | fc2t | ones-row [1,16]
        FF = 2 * N + C + 16
        blobF = np.zeros((16, FF), np.float32)
        for b in range(B):
            blobF[:, b * N: (b + 1) * N] = np.broadcast_to(
                keep[b][None, :], (C, N))
        blobF[:, 2 * N: 2 * N + C] = fc2t
        blobF[:, 2 * N + C: 2 * N + C + 16] = 1.0    # ones [16,16] block

        # fp16 blobH rides in blobB's tail as raw bytes (both 2-byte)
        import ml_dtypes
        blobBH = np.concatenate(
            [_bf16(blobB), _fp16(blobH).view(ml_dtypes.bfloat16)], axis=1)
        in_maps.append(dict(blobB=blobBH, blobF=blobF, cbP=cbP))
    return in_maps


# ----------------------------------------------------------------------
# device program
# ----------------------------------------------------------------------

def _build_nc(reps=1, pair_r2=True, pool_tail=True):
    nc = bacc.Bacc("TRN2", target_bir_lowering=False, debug=False,
                   num_devices=N_CORES)
    FH = 2 * MP + 4 * DH * C
    FB = 2 * 512 + FH
    FF = 2 * N + C + 16
    d_blobB = nc.dram_tensor("blobB", [18, FB], BF16, kind="ExternalInput")
    d_blobF = nc.dram_tensor("blobF", [16, FF], F32, kind="ExternalInput")
    d_cbP = nc.dram_tensor("cbP", [128, 2 * DH], F32, kind="ExternalInput")
    d_out = nc.dram_tensor("out", [C, B], F32, kind="ExternalOutput")

    groups = [list(range(N_CORES))]

    with tile.TileContext(nc) as tc:
        with (
            tc.tile_pool(name="const", bufs=2) as cpool,
            tc.tile_pool(name="bas", bufs=2) as baspool,
            tc.tile_pool(name="work", bufs=2) as wpool,
            tc.tile_pool(name="tiny", bufs=2) as tpool,
            tc.tile_pool(name="psum", bufs=2, space="PSUM") as psum,
            tc.tile_pool(name="psumt", bufs=1, space="PSUM") as psumt,
            tc.tile_pool(name="dram", bufs=1, space="DRAM") as dram,
        ):
            for _rep in range(reps):
                rep_out = d_out if _rep == reps - 1 else dram.tile(
                    [C, B], F32, tag="outscratch")

                # --- constant/input blobs: 4 DMAs ---
                blobB = cpool.tile([18, FB], BF16, tag="blobB")
                nc.sync.dma_start(out=blobB[:], in_=d_blobB[:])
                blobF = cpool.tile([16, FF], F32, tag="blobF")
                nc.sync.dma_start(out=blobF[:], in_=d_blobF[:])
                cbP = cpool.tile([128, 2 * DH], F32, tag="cbP")
                nc.sync.dma_start(out=cbP[:], in_=d_cbP[:])

                def glh_b(b):
                    return blobB[0:KG, b * 512: b * 512 + 128]

                def grh_b(b):
                    return blobB[0:KG, b * 512 + 128: (b + 1) * 512]

                def x0_b(b):
                    return blobB[0:C, 1024 + b * MP:
                                 1024 + (b + 1) * MP].bitcast(FP16)

                def qeo(l, par):
                    o = 1024 + 2 * MP + (2 * l + par) * DH * C
                    return blobB[0:C, o: o + DH * C].bitcast(FP16)

                def keep_b(b):
                    return blobF[:, b * N: (b + 1) * N]

                fc2t = blobF[:, 2 * N: 2 * N + C]
                ones16 = blobF[:, 2 * N + C: 2 * N + C + C]

                rs_in = [dram.tile([N_CORES, C, MS], F32, tag=f"rsin{b}",
                                   name=f"rsin{b}_{_rep}") for b in range(B)]
                rs_out = [dram.tile([C, MS], F32, tag=f"rsout{b}",
                                    name=f"rsout{b}_{_rep}") for b in range(B)]

                def make_z(l, xt_sb, ps_z):
                    nc.tensor.matmul(ps_z[0:MP, :], xt_sb, qeo(l, 0),
                                     start=True, stop=True)
                    nc.tensor.matmul(ps_z[MP:128, :], xt_sb, qeo(l, 1),
                                     start=True, stop=True,
                                     tile_position=(0, MP))
                    zsb = wpool.tile([128, DH, C], FP16, tag="zsb")
                    nc.vector.tensor_copy(zsb[:], ps_z[:])
                    return zsb

                def conv_mms(ps_c, zsb, bas, b):
                    for t in range(DH):
                        nc.tensor.matmul(ps_c[:], zsb[:, t, :],
                                         bas[:, t, b, :],
                                         start=(t == 0), stop=(t == DH - 1))

                # --- r^2 for both b in one PSUM pair-tile; basis for both b
                #     per single ACT instruction ---
                if pair_r2:
                    ps_r2 = psum.tile([128, B, 512], F32, tag="ps_r2")
                    for b in range(B):
                        nc.tensor.matmul(ps_r2[:, b, 0:N], glh_b(b), grh_b(b),
                                         start=True, stop=True)
                    bas = baspool.tile([128, DH, B, N], FP16, tag="bas")
                    for t in range(DH):
                        nc.scalar.activation(bas[:, t, :, :], ps_r2[:, :, 0:N],
                                             AF.Derivative_Erf,
                                             scale=cbP[:, t:t + 1],
                                             bias=cbP[:, DH + t:DH + t + 1])
                    bas_b = bas
                else:
                    bas_b = baspool.tile([128, DH, B, N], FP16, tag="bas")
                    for b in range(B):
                        ps_r2 = psum.tile([128, N], F32, tag="ps_r2")
                        nc.tensor.matmul(ps_r2[:], glh_b(b), grh_b(b),
                                         start=True, stop=True)
                        for t in range(DH):
                            nc.scalar.activation(bas_b[:, t, b, :], ps_r2[:],
                                                 AF.Derivative_Erf,
                                                 scale=cbP[:, t:t + 1],
                                                 bias=cbP[:, DH + t:DH + t + 1])

                # --- conv1 + per-b ReduceScatter ---
                for b in range(B):
                    ps_zc = psum.tile([128, 512], F32, tag="ps_zc")
                    z1 = make_z(0, x0_b(b), ps_zc[:, 0:DH * C])
                    ps_c1 = ps_zc[0:C, 128:512]
                    conv_mms(ps_c1, z1, bas_b, b)
                    x1p = wpool.tile([C, N], F32, tag="x1p")
                    nc.vector.tensor_copy(x1p[:], ps_c1)
                    nc.sync.dma_start(
                        out=rs_in[b][:].rearrange("c i m -> i c m"),
                        in_=x1p[:].rearrange("i (c m) -> i c m", c=N_CORES))
                    nc.gpsimd.collective_compute(
                        "ReduceScatter", ALU.add, replica_groups=groups,
                        ins=[rs_in[b].opt()], outs=[rs_out[b].opt()])

                # --- conv2 on the scattered slice + masked reduce ---
                s_b = tpool.tile([C, B], F32, tag="sb")
                for b in range(B):
                    x1t = wpool.tile([C, MP], FP16, tag="x1t")
                    nc.vector.memset(x1t[:], 0.0)
                    x1f = wpool.tile([C, MS], F32, tag="x1f")
                    nc.sync.dma_start(out=x1f[:], in_=rs_out[b][:])
                    nc.vector.tensor_copy(x1t[:, 0:MS], x1f[:])
                    ps_zc = psum.tile([128, 512], F32, tag="ps_zc")
                    z2 = make_z(1, x1t[:], ps_zc[:, 0:DH * C])
                    ps_c2 = ps_zc[0:C, 128:512]
                    conv_mms(ps_c2, z2, bas_b, b)
                    xm2 = wpool.tile([C, N], F32, tag="xm2")
                    nc.vector.tensor_tensor(out=xm2[:], in0=ps_c2,
                                            in1=keep_b(b), op=ALU.mult)
                    nc.vector.reduce_sum(s_b[:, b:b + 1], xm2[:], axis=AX.X)

                ar_in = dram.tile([C, B], F32, tag="arin", name=f"arin_{_rep}")
                nc.sync.dma_start(out=ar_in[:], in_=s_b[:])
                ar_out = dram.tile([C, B], F32, tag="arout",
                                   name=f"arout_{_rep}")
                nc.gpsimd.collective_compute(
                    "AllReduce", ALU.add, replica_groups=groups,
                    ins=[ar_in.opt()], outs=[ar_out.opt()])

                # --- tail in [C, B] layout: normalize (ddof=1) + fc2 +
                #     softmax.  ones[16,16] matmuls fuse partition-reduce +
                #     broadcast; PSUM-reading elementwise ops go to DVE,
                #     SBUF-only ones to Pool; rsqrt = bit trick + 1 Newton,
                #     exp = (1 + t + t^2/2)^256 by repeated squaring ---
                st = tpool.tile([C, B], F32, tag="st")
                nc.sync.dma_start(out=st[:], in_=ar_out[:])
                ps_mu = psumt.tile([C, B], F32, tag="tailCB")
                nc.tensor.matmul(ps_mu[:], ones16, st[:], start=True,
                                 stop=True)
                cen = tpool.tile([C, B], F32, tag="cen")
                nc.vector.tensor_scalar(out=cen[:], in0=ps_mu[:],
                                        scalar1=-1.0 / C, scalar2=None,
                                        op0=ALU.mult)
                nc.gpsimd.tensor_tensor(out=cen[:], in0=cen[:], in1=st[:],
                                        op=ALU.add)
                sq2 = tpool.tile([C, B], F32, tag="sq2")
                nc.gpsimd.tensor_tensor(out=sq2[:], in0=cen[:], in1=cen[:],
                                        op=ALU.mult)
                ps_vs = psumt.tile([C, B], F32, tag="tailCB")
                nc.tensor.matmul(ps_vs[:], ones16, sq2[:], start=True,
                                 stop=True)

                # rinv = sqrt(15)/sqrt(varsum), rows all equal
                sh = tpool.tile([C, B], I32, tag="sh")
                nc.vector.tensor_scalar(out=sh[:], in0=ps_vs[:].bitcast(I32),
                                        scalar1=1, scalar2=None,
                                        op0=ALU.logical_shift_right)
                nsh = tpool.tile([C, B], I32, tag="nsh")
                nc.vector.tensor_scalar(out=nsh[:], in0=sh[:],
                                        scalar1=-1, scalar2=None,
                                        op0=ALU.bitwise_xor)
                z0i = tpool.tile([C, B], I32, tag="z0i")
                nc.vector.tensor_scalar(out=z0i[:], in0=nsh[:],
                                        scalar1=RSQRT_MAGIC + 1, scalar2=None,
                                        op0=ALU.add)
                z0 = z0i[:].bitcast(F32)
                t1 = tpool.tile([C, B], F32, tag="t1")
                nc.vector.tensor_tensor(out=t1[:], in0=z0, in1=z0, op=ALU.mult)
                t2 = tpool.tile([C, B], F32, tag="t2")
                nc.vector.tensor_tensor(out=t2[:], in0=t1[:], in1=ps_vs[:],
                                        op=ALU.mult)
                h = tpool.tile([C, B], F32, tag="h")
                nc.vector.tensor_scalar(out=h[:], in0=t2[:],
                                        scalar1=-0.5 * SQ15, scalar2=1.5 * SQ15,
                                        op0=ALU.mult, op1=ALU.add)
                rinv = tpool.tile([C, B], F32, tag="rinv")
                nc.vector.tensor_tensor(out=rinv[:], in0=z0, in1=h[:],
                                        op=ALU.mult)
                normed = tpool.tile([C, B], F32, tag="normed")
                nc.gpsimd.tensor_tensor(out=normed[:], in0=cen[:],
                                        in1=rinv[:], op=ALU.mult)

                ps_l = psumt.tile([C, B], F32, tag="tailCB")
                nc.tensor.matmul(ps_l[:], fc2t, normed[:], start=True,
                                 stop=True)

                te = tpool.tile([C, B], F32, tag="te")
                nc.vector.tensor_scalar(out=te[:], in0=ps_l[:],
                                        scalar1=1.0 / 256.0, scalar2=None,
                                        op0=ALU.mult)
                tsq = tpool.tile([C, B], F32, tag="tsq")
                nc.gpsimd.tensor_tensor(out=tsq[:], in0=te[:], in1=te[:],
                                        op=ALU.mult)
                u = tpool.tile([C, B], F32, tag="u")
                nc.gpsimd.tensor_scalar(out=u[:], in0=tsq[:], scalar1=0.5,
                                        scalar2=1.0, op0=ALU.mult,
                                        op1=ALU.add)
                el = tpool.tile([C, B], F32, tag="el")
                nc.gpsimd.tensor_tensor(out=el[:], in0=u[:], in1=te[:],
                                        op=ALU.add)
                for _ in range(8):
                    nc.gpsimd.tensor_tensor(out=el[:], in0=el[:], in1=el[:],
                                            op=ALU.mult)

                ps_den = psumt.tile([C, B], F32, tag="tailCB")
                nc.tensor.matmul(ps_den[:], ones16, el[:], start=True,
                                 stop=True)
                rden = tpool.tile([C, B], F32, tag="rden")
                nc.vector.reciprocal(rden[:], ps_den[:])
                outf = tpool.tile([C, B], F32, tag="outf")
                nc.vector.tensor_tensor(out=outf[:], in0=el[:], in1=rden[:],
                                        op=ALU.mult)
                nc.sync.dma_start(out=rep_out[:], in_=outf[:])

    nc.compile()
    return nc


def get_nc(reps=1, **kw):
    key = ("nc", reps, tuple(sorted(kw.items())))
    if key not in _CACHE:
        _CACHE[key] = _build_nc(reps, **kw)
    return _CACHE[key]


def kernel(x, xyz, mask, conv1_w1, conv1_w2, conv2_w1, conv2_w2, fc2_w,
           _return_results=False, **_unused):
    nc = get_nc()
    in_maps = _host_prep(x, xyz, mask, conv1_w1, conv1_w2,
                         conv2_w1, conv2_w2, fc2_w)
    res = None
    last_err = None
    for attempt in range(4):
        try:
            res = run_bass_kernel_spmd(nc, in_maps,
                                       core_ids=list(range(N_CORES)))
            break
        except Exception as e:  # transient NRT/axon wedges recover in ~10-30s
            last_err = e
            time.sleep(10.0 * (attempt + 1))
    if res is None:
        raise last_err
    if _return_results:
        return res
    return np.asarray(res.results[0]["out"], np.float32).T.copy()
